# revision 29
# baseline (speedup 1.0000x reference)
"""DPLR SSM block kernel for Trainium2 (8 NeuronCores, batch-sharded).

Math: A = diag(d) + p q^T  (diagonal + rank-1).  The reference computes
    expA  = expm(delta A)
    B_disc = pinv(A) (expA - I) b_mat
    out   = h expA^T + x B_disc^T

Structure exploited here (all small factors computed on host in f64):
  * Duhamel: expm(delta A) = e^{delta D} + int_0^delta e^{(delta-s)D} p w(s)^T ds
    with w(t) = expm(t A^T) q solving w' = D w + q (p.w)  (O(H) per RK4 step).
    Gauss-Legendre quadrature (NQ nodes) makes the correction rank-NQ:
        F = diag(ed) + A_mat @ W
  * Van Loan: A^{-1}(e^{dA} - I) = int_0^delta e^{sA} ds
        G = diag(gd) + B_mat @ W            (same W, different outer weights)
  * pinv truncation: jax pinv zeroes singular values < 10*H*eps32*sigma_max.
    The truncated pairs (few) are found via subspace iteration on
    (A^T A)^{-1} using O(H) Sherman-Morrison solves, giving a small extra
    rank correction  -Vtil @ R with R = U_t^T (F - I).

The device then only computes (per core, batch slice I = B/8, transposed):
    YT   = b_mat @ xT_s                  -- the single dense HxHxI matmul
    V1   = W @ hT_s ; V2 = [W; R] @ YT   -- tiny rank-K GEMMs
    outT = ed (.) hT + gd (.) YT + A_mat V1 + [B_mat | -Vtil] V2

The big matmul runs in bf16 (half the DMA traffic, full PE rate, errors well
under tolerance); the small correction GEMMs run in float32r (FP22 multiplies,
fp32 accumulate).  Validated against the f32 jax reference on hardware:
absmax err ~1.4e-3 on out_scale 4.6 (relative error ~3e-4).
"""

import sys

import numpy as np

sys.path.insert(0, "/opt/trn_rl_repo")

H = 2048
B = 2048
N_CORES = 8
I_PER_CORE = B // N_CORES  # 256
NQ = 16                    # Gauss-Legendre quadrature nodes
P = 128                    # partitions
KT = H // P                # 16 k/j tiles

_cache = {}


# --------------------------------------------------------------------------
# host-side prep (float64, all O(H * small))
# --------------------------------------------------------------------------

def _phi(tau, d):
    """(exp(tau*d)-1)/d elementwise, safe at d ~ 0."""
    tau = np.asarray(tau, np.float64)
    small = np.abs(d) < 1e-30
    dd = np.where(small, 1.0, d)
    return np.where(small, tau, np.expm1(tau * d) / dd)


def _host_factors(d, p, q, delta):
    rng = np.random.default_rng(12345)

    nodes, wts = np.polynomial.legendre.leggauss(NQ)
    nodes = 0.5 * delta * (nodes + 1.0)
    wts = 0.5 * delta * wts

    # ---- W[j] = w(s_j)^T,  w' = D w + q (p.w), w(0) = q  (RK4) ----
    W = np.zeros((NQ, H))
    w = q.copy()
    t = 0.0
    scale = max(1.0, float(np.abs(d).max() * delta),
                float(np.linalg.norm(p) * np.linalg.norm(q) * delta))
    f = lambda v: d * v + q * (p @ v)
    for j, s in enumerate(nodes):
        seg = s - t
        nst = max(8, int(np.ceil(120.0 * scale * seg / max(delta, 1e-300))))
        hs = seg / nst
        for _ in range(nst):
            k1 = f(w)
            k2 = f(w + 0.5 * hs * k1)
            k3 = f(w + 0.5 * hs * k2)
            k4 = f(w + hs * k3)
            w = w + (hs / 6.0) * (k1 + 2.0 * k2 + 2.0 * k3 + k4)
        t = s
        W[j] = w

    ed = np.exp(delta * d)
    gd = _phi(delta, d)
    A_mat = wts[None, :] * np.exp((delta - nodes)[None, :] * d[:, None]) * p[:, None]
    B_mat = wts[None, :] * _phi((delta - nodes)[None, :], d[:, None]) * p[:, None]

    # ---- pinv truncation emulation ----
    # Sherman-Morrison applies for A = D + p q^T
    dinv = 1.0 / np.where(np.abs(d) < 1e-300, 1e-300, d)
    c_sm = 1.0 + q @ (dinv * p)

    def A_apply(z):
        if z.ndim == 1:
            return d * z + p * (q @ z)
        return (d * z.T).T + np.outer(p, q @ z)

    def AT_apply(z):
        if z.ndim == 1:
            return d * z + q * (p @ z)
        return (d * z.T).T + np.outer(q, p @ z)

    R = np.zeros((0, H))
    Vtil = np.zeros((H, 0))
    if abs(c_sm) > 1e-12:  # A invertible: emulate pinv's small-sv truncation
        def A_solve(z):
            t0 = (dinv * z.T).T
            return t0 - np.outer(dinv * p, q @ t0) / c_sm

        def AT_solve(z):
            t0 = (dinv * z.T).T
            return t0 - np.outer(dinv * q, p @ t0) / c_sm

        v = rng.standard_normal(H)
        v /= np.linalg.norm(v)
        for _ in range(200):
            v = AT_apply(A_apply(v))
            v /= np.linalg.norm(v)
        sigma_max = float(np.sqrt(v @ AT_apply(A_apply(v))))
        cutoff = 10.0 * H * np.finfo(np.float32).eps * sigma_max

        m = 24
        Z = rng.standard_normal((H, m))
        for _ in range(80):
            Z = A_solve(AT_solve(Z))
            Z, _ = np.linalg.qr(Z)
        lam, Q2 = np.linalg.eigh(Z.T @ AT_apply(A_apply(Z)))
        Vr = Z @ Q2
        sig = np.sqrt(np.maximum(lam, 0.0))
        ntr = int((sig < cutoff).sum())
        ntr = min(ntr, m - 4)
        if ntr > 0:
            Vs = Vr[:, :ntr]                       # right singular vectors
            ss = sig[:ntr]
            Us = A_apply(Vs) / ss[None, :]         # left singular vectors
            # r_i^T = u_i^T (F - I);  F = diag(ed) + A_mat @ W
            R = (Us.T * ed[None, :]) - Us.T + (Us.T @ A_mat) @ W   # [ntr, H]
            Vtil = Vs / ss[None, :]                                # [H, ntr]

    return W, ed, gd, A_mat, B_mat, R, Vtil


# --------------------------------------------------------------------------
# device program
# --------------------------------------------------------------------------

def _build_program(kr, reps=1, phases=3):
    """kr = number of extra truncation-correction rows (R rows).
    reps > 1 repeats the whole (idempotent) body for slope-timing.
    phases: 0 loads only, 1 +Y loop, 2 +V1/V2, 3 full (diagnostics)."""
    import concourse.bacc as bacc
    import concourse.mybir as mybir
    import concourse.tile as tile
    from concourse.bass import MemorySpace

    f32 = mybir.dt.float32
    f32r = mybir.dt.float32r
    bf16 = mybir.dt.bfloat16
    K2 = NQ + kr  # rows of L2 / V2

    nc = bacc.Bacc("TRN2", target_bir_lowering=False, debug=False)

    mult = mybir.AluOpType.mult
    add = mybir.AluOpType.add

    # all host-side arrays are pre-tiled so every DMA is fully contiguous
    bT_d = nc.dram_tensor("bT", [KT, P, KT, P], bf16, kind="ExternalInput").ap()
    xT_d = nc.dram_tensor("xT", [P, KT, I_PER_CORE], bf16, kind="ExternalInput").ap()
    hT_d = nc.dram_tensor("hT", [P, KT, I_PER_CORE], f32, kind="ExternalInput").ap()
    Wt_d = nc.dram_tensor("Wt", [P, KT, NQ], f32, kind="ExternalInput").ap()
    L2_d = nc.dram_tensor("L2", [P, KT, K2], f32, kind="ExternalInput").ap()
    U1T_d = nc.dram_tensor("U1T", [NQ, H], f32, kind="ExternalInput").ap()
    U2T_d = nc.dram_tensor("U2T", [K2, H], f32, kind="ExternalInput").ap()
    ed_d = nc.dram_tensor("ed2", [P, KT], f32, kind="ExternalInput").ap()
    gd_d = nc.dram_tensor("gd2", [P, KT], f32, kind="ExternalInput").ap()
    outT_d = nc.dram_tensor("outT", [H, I_PER_CORE], f32, kind="ExternalOutput").ap()

    with tile.TileContext(nc) as tc:
        with (
            tc.tile_pool(name="const", bufs=1) as cpool,
            tc.tile_pool(name="bstream", bufs=6) as bpool,
            tc.tile_pool(name="ypsum", bufs=3, space=MemorySpace.PSUM) as ypool,
            tc.tile_pool(name="vpsum", bufs=1, space=MemorySpace.PSUM) as vpool,
            tc.tile_pool(name="cpsum", bufs=3, space=MemorySpace.PSUM) as corrpool,
            tc.tile_pool(name="osb", bufs=3) as opool,
        ):
            # b and x feed only the big matmul: bf16 (half DMA, same PE rate).
            # Everything else stays f32/f32r.  f32r tiles are DMA'd as bitcast
            # views of f32 DRAM (walrus accepts f32r-ML DMACopy producers).
            xT_sb = cpool.tile([P, KT, I_PER_CORE], bf16)
            nc.sync.dma_start(xT_sb[:], xT_d[:])
            hT_sb = cpool.tile([P, KT, I_PER_CORE], f32r)
            nc.sync.dma_start(hT_sb[:], hT_d.bitcast(f32r))
            Wt_sb = cpool.tile([P, KT, NQ], f32r)
            nc.sync.dma_start(Wt_sb[:], Wt_d.bitcast(f32r))
            L2_sb = cpool.tile([P, KT, K2], f32r)
            nc.sync.dma_start(L2_sb[:], L2_d.bitcast(f32r))

            YT_sb = cpool.tile([P, KT, I_PER_CORE], f32r)

            U1T_sb = cpool.tile([NQ, H], f32r)
            U2T_sb = cpool.tile([K2, H], f32r)
            ed_sb = cpool.tile([P, KT], f32)
            nc.sync.dma_start(ed_sb[:], ed_d[:])
            gd_sb = cpool.tile([P, KT], f32)
            nc.sync.dma_start(gd_sb[:], gd_d[:])
            egsum_sb = cpool.tile([P, KT, I_PER_CORE], f32)

            for rep in range(reps):
                if phases < 1:
                    break
                # ---- YT = b_mat @ xT (bt streamed, fully contiguous DMAs) ----
                # v2 accumulates inside the loop as each YT tile lands
                v2psum = vpool.tile([K2, I_PER_CORE], f32, tag="v2")
                for jp2 in range(KT // 2):
                    bt_tile = bpool.tile([P, 2, KT, P], bf16, tag="bt")
                    nc.sync.dma_start(
                        bt_tile[:],
                        bT_d[jp2 * 2:jp2 * 2 + 2].rearrange(
                            "two kk kt j -> kk two kt j"))
                    for half in range(2):
                        jt = jp2 * 2 + half
                        ypsum = ypool.tile([P, I_PER_CORE], f32, tag="yp")
                        for kt in range(KT):
                            nc.tensor.matmul(
                                ypsum[:],
                                bt_tile[:, half, kt, :],
                                xT_sb[:, kt, :],
                                start=(kt == 0),
                                stop=(kt == KT - 1),
                            )
                        nc.vector.tensor_copy(YT_sb[:, jt, :], ypsum[:])
                        nc.vector.tensor_scalar_mul(
                            egsum_sb[:, jt, :], hT_sb[:, jt, :].bitcast(f32),
                            ed_sb[:, jt:jt + 1],
                        )
                        nc.vector.scalar_tensor_tensor(
                            egsum_sb[:, jt, :], YT_sb[:, jt, :].bitcast(f32),
                            gd_sb[:, jt:jt + 1], egsum_sb[:, jt, :], mult, add,
                        )

                # ---- V2 = [W; R] @ YT ----
                if phases < 2:
                    continue
                for jt in range(KT):
                    nc.tensor.matmul(
                        v2psum[:],
                        L2_sb[:, jt, :],
                        YT_sb[:, jt, :],
                        start=(jt == 0),
                        stop=(jt == KT - 1),
                    )

                nc.sync.dma_start(U1T_sb[:], U1T_d.bitcast(f32r))
                nc.sync.dma_start(U2T_sb[:], U2T_d.bitcast(f32r))

                # ---- V1 = W @ hT ----
                v1psum = vpool.tile([NQ, I_PER_CORE], f32, tag="v1")
                for kt in range(KT):
                    nc.tensor.matmul(
                        v1psum[:],
                        Wt_sb[:, kt, :],
                        hT_sb[:, kt, :],
                        start=(kt == 0),
                        stop=(kt == KT - 1),
                    )
                v1_sb = cpool.tile([NQ, I_PER_CORE], f32r, tag="v1s")
                nc.vector.tensor_copy(v1_sb[:], v1psum[:])
                v2_sb = cpool.tile([K2, I_PER_CORE], f32r, tag="v2s")
                nc.vector.tensor_copy(v2_sb[:], v2psum[:])

                # ---- outT = ed.hT + gd.YT + U1 V1 + U2 V2 ----
                if phases < 3:
                    continue
                for jp in range(KT // 4):
                    out_sb = opool.tile([P, 4, I_PER_CORE], f32, tag="ot")
                    for half in range(4):
                        jt = jp * 4 + half
                        cpsum = corrpool.tile([P, I_PER_CORE], f32, tag="cp")
                        nc.tensor.matmul(
                            cpsum[:],
                            U1T_sb[:, jt * P:(jt + 1) * P],
                            v1_sb[:],
                            start=True,
                            stop=False,
                        )
                        nc.tensor.matmul(
                            cpsum[:],
                            U2T_sb[:, jt * P:(jt + 1) * P],
                            v2_sb[:],
                            start=False,
                            stop=True,
                        )
                        nc.vector.tensor_tensor(
                            out_sb[:, half, :], cpsum[:], egsum_sb[:, jt, :],
                            mybir.AluOpType.add,
                        )
                    nc.sync.dma_start(
                        outT_d[jp * 4 * P:(jp + 1) * 4 * P, :].rearrange(
                            "(four kk) i -> kk four i", kk=P),
                        out_sb[:])

    nc.compile()
    return nc


def _make_in_maps(h, x, b_mat, W, ed, gd, A_mat, B_mat, R, Vtil):
    """Pre-tile all arrays into the exact on-device layouts (contiguous DMA)."""
    import ml_dtypes
    f32 = np.float32
    bf16 = ml_dtypes.bfloat16

    bT = np.ascontiguousarray(b_mat.T)                # [k, jcol]
    # BT4[jt, kk, kt, j] = bT[kt*P+kk, jt*P+j]
    BT4 = np.ascontiguousarray(
        bT.reshape(KT, P, KT, P).transpose(2, 1, 0, 3).astype(bf16))

    def part_tile(a2d):                               # [H, I] -> [P, KT, I]
        return np.ascontiguousarray(
            a2d.reshape(KT, P, a2d.shape[1]).transpose(1, 0, 2))

    xT = np.ascontiguousarray(x.T)
    hT = np.ascontiguousarray(h.T)
    Wt3 = part_tile(np.ascontiguousarray(W.T, dtype=f32))
    L23 = part_tile(np.ascontiguousarray(
        np.concatenate([W.T, R.T], axis=1), dtype=f32))
    U1T = np.ascontiguousarray(A_mat.T, dtype=f32)
    U2T = np.ascontiguousarray(np.concatenate([B_mat, -Vtil], axis=1).T, dtype=f32)
    ed2 = np.ascontiguousarray(ed.reshape(KT, P).T, dtype=f32)
    gd2 = np.ascontiguousarray(gd.reshape(KT, P).T, dtype=f32)

    in_maps = []
    for c in range(N_CORES):
        sl = slice(c * I_PER_CORE, (c + 1) * I_PER_CORE)
        in_maps.append({
            "bT": BT4,
            "xT": part_tile(xT[:, sl]).astype(bf16),
            "hT": part_tile(hT[:, sl].astype(f32)),
            "Wt": Wt3,
            "L2": L23,
            "U1T": U1T,
            "U2T": U2T,
            "ed2": ed2,
            "gd2": gd2,
        })
    return in_maps


# --------------------------------------------------------------------------
# entry point
# --------------------------------------------------------------------------

def kernel(h, x, a_diag, p_vec, q_vec, b_mat, delta):
    from concourse import bass_utils

    h = np.asarray(h, np.float32)
    x = np.asarray(x, np.float32)
    b_mat = np.asarray(b_mat, np.float32)
    d = np.asarray(a_diag, np.float64).ravel()
    p = np.asarray(p_vec, np.float64).ravel()
    q = np.asarray(q_vec, np.float64).ravel()
    dl = float(np.asarray(delta, np.float64).ravel()[0])

    W, ed, gd, A_mat, B_mat, R, Vtil = _host_factors(d, p, q, dl)
    kr = R.shape[0]

    if ("prog", kr) not in _cache:
        _cache[("prog", kr)] = _build_program(kr)
    nc = _cache[("prog", kr)]

    in_maps = _make_in_maps(h, x, b_mat, W, ed, gd, A_mat, B_mat, R, Vtil)

    res = bass_utils.run_bass_kernel_spmd(nc, in_maps, list(range(N_CORES)))

    out = np.empty((B, H), dtype=np.float32)
    for c in range(N_CORES):
        out[c * I_PER_CORE:(c + 1) * I_PER_CORE, :] = res.results[c]["outT"].T
    return out

